# revision 11
# baseline (speedup 1.0000x reference)
"""Trainium2 Bass kernel for ViTDet-style attention with decomposed
relative-position bias.

Problem shapes (hardcoded):
  x: (4, 32, 32, 768) f32, Wqkv: (768, 2304), Wproj: (768, 768),
  bproj: (768,), rel_pos_h/w: (63, 64).
  12 heads, head_dim 64, S = 32*32 = 1024.

Sharding: 48 (batch, head) pairs -> 6 heads per core, all of one batch per
core-pair. Each core computes its heads' attention and a partial output
projection (its heads' channel rows of Wproj); the host sums the two
partials per batch and adds bproj.

Device algorithm per core:
  - qkT = Wqk^T @ x^T and v = x @ Wv. With FP8_QKV these use fp8e4
    DoubleRow (two 128-row K tiles per pass, 2x); otherwise bf16.
    Weights are pre-scaled x64 (k additionally by 1/sqrt(hd)) so fp8 stays
    in normal range; the exp activation's scale=2^-12 folds it back out.
  - rel-pos bias band rows computed DIRECTLY per shift: for query row h the
    band tile rows are rhT_flip[:, 31-h:63-h]^T @ q64 — no 63-row table
    intermediate. Two heads per matmul (a matmul's PSUM extent must stay
    inside one 2KB bank), four 256-col quarters per axis.
  - scoresT (k x q) = kaug^T @ qaug in one K=128 bf16 matmul per tile:
    rows 0-63 k8/q64, 64-95 one-hot(kh)/bandH, 96-127 one-hot(kw)/bandW.
  - e = exp(2^-12 * scores) on ScalarE; av accumulates over k blocks with
    v blocks padded to 128 stationary cols (64 v + ones + 63 zeros); row 64
    of av is the softmax denominator via the ones column.
  - normalize: av->SBUF f32 copy (frees PSUM fast), DVE
    reciprocal_approx_fast on the denominator row (plain reciprocal is ~9
    cyc/elem and single-lane here), gpsimd partition-broadcast, DVE mult.
  - partial = out_headsT^T @ (Wproj/64) in bf16, PSUM->SBUF->DRAM.
"""

import numpy as np

import concourse.bass as bass
import concourse.bacc as bacc
import concourse.mybir as mybir
import concourse.tile as tile
from concourse.bass_utils import run_bass_kernel_spmd

F32 = mybir.dt.float32
BF16 = mybir.dt.bfloat16
FP8 = mybir.dt.float8e4

NH = 12          # total heads
C = 768
HD = 64
H = W = 32
S = H * W        # 1024
B = 4
NCORES = 8
HPC = NH * B // NCORES   # heads per core = 6
EXP_SCALE = float(2.0 ** -12)

FP8_QKV = False  # fp8 DoubleRow for the qk/v projections (adds ~1.7e-2 err)


def _ap(t, off, dims):
    return bass.AP(t.tensor, t[:].offset + off, [t[:].ap[0]] + dims)


def _app(t, p0, psz, off, dims):
    # AP with partition offset/size override
    base = t[p0:p0 + psz, :]
    return bass.AP(t.tensor, base.offset + off, [base.ap[0]] + dims)


def build_program():
    nc = bacc.Bacc("TRN2", target_bir_lowering=False, debug=False)

    XDT = FP8 if FP8_QKV else BF16
    xTp = nc.declare_dram_parameter("xTp", [128, 6144], XDT, isOutput=False)
    wqkp = nc.declare_dram_parameter("wqkp", [128, 4608], XDT, isOutput=False)
    wvp = nc.declare_dram_parameter("wvp", [128, 2304], XDT, isOutput=False)
    wpr = nc.declare_dram_parameter("wpr", [384, 768], BF16, isOutput=False)
    rh2 = nc.declare_dram_parameter("rh2", [64, 63], BF16, isOutput=False)
    rw2 = nc.declare_dram_parameter("rw2", [64, 63], BF16, isOutput=False)
    ohk = nc.declare_dram_parameter("ohk", [64, 1024], BF16, isOutput=False)
    out = nc.declare_dram_parameter("out", [S, C], F32, isOutput=True)

    with tile.TileContext(nc) as tc:
        with (
            tc.tile_pool(name="persist", bufs=1) as persist,
            tc.tile_pool(name="ps", bufs=2, space="PSUM") as ps,
            tc.tile_pool(name="small", bufs=2) as small,
        ):
            # ---- persistent SBUF loads (order = need order) ----
            xTp_sb = persist.tile([128, 6144], XDT, tag="xTp", name="xTp_sb")
            nc.sync.dma_start(xTp_sb[:], xTp[:, :])
            wqkp_sb = persist.tile([128, 4608], XDT, tag="wqkp", name="wqkp_sb")
            nc.sync.dma_start(wqkp_sb[:], wqkp[:, :])
            rh2_sb = persist.tile([64, 63], BF16, tag="rh2", name="rh2_sb")
            nc.sync.dma_start(rh2_sb[:], rh2[:, :])
            rw2_sb = persist.tile([64, 63], BF16, tag="rw2", name="rw2_sb")
            nc.sync.dma_start(rw2_sb[:], rw2[:, :])

            # qaug/kaug: per head i at cols [1024i, 1024(i+1)):
            #   rows 0-63 q64 / k8, 64-95 bandH / onehot(kh), 96-127 bandW /
            #   onehot(kw)
            qaug = persist.tile([128, HPC * S], BF16, tag="qaug", name="qaug")
            kaug = persist.tile([128, HPC * S], BF16, tag="kaug", name="kaug")
            for i in range(HPC):
                nc.sync.dma_start(kaug[64:128, S * i:S * (i + 1)], ohk[:, :])

            wvp_sb = persist.tile([128, 2304], XDT, tag="wvp", name="wvp_sb")
            nc.sync.dma_start(wvp_sb[:], wvp[:, :])
            wpr_sb = persist.tile([128, 2304], BF16, tag="wpr", name="wpr_sb")
            nc.sync.dma_start(
                wpr_sb[:],
                bass.AP(wpr, 0, [[768, 128], [128 * 768, 3], [1, 768]]))

            # v in bf16 pair layout: vps[p, j, i, h, d] with j = k-block pair,
            # i = which block of the pair, h = head, d = 128 cols
            # (64 v + ones + 63 zeros)
            vps = persist.tile([128, 4 * 2 * HPC * 128], BF16, tag="vps",
                               name="vps")
            ohT = persist.tile([128, 3 * S], BF16, tag="ohT", name="ohT")

            DR = mybir.MatmulPerfMode.DoubleRow

            # ---- qk projection ----
            def qk_oct(t):
                # octile t: rows 128t..128t+128 of qk output; t<3 -> q64,
                # t>=3 -> k8; heads (2(t%3), 2(t%3)+1)
                qp = ps.tile([128, S], F32, tag="big", name="qp")
                if FP8_QKV:
                    for j in range(3):
                        for nh in range(2):
                            sl = 512 * nh
                            nc.tensor.matmul(
                                qp[:, sl:sl + 512],
                                _ap(wqkp_sb, 1536 * j + 128 * t,
                                    [[768, 2], [1, 128]]),
                                _ap(xTp_sb, 2048 * j + sl, [[1024, 2], [1, 512]]),
                                start=(j == 0), stop=(j == 2), perf_mode=DR)
                else:
                    for ci in range(6):
                        for nh in range(2):
                            sl = 512 * nh
                            nc.tensor.matmul(
                                qp[:, sl:sl + 512],
                                wqkp_sb[:, 768 * ci + 128 * t:
                                        768 * ci + 128 * (t + 1)],
                                xTp_sb[:, 1024 * ci + sl:1024 * ci + sl + 512],
                                start=(ci == 0), stop=(ci == 5))
                for sub in range(2):
                    head = (t % 3) * 2 + sub
                    dst = (qaug if t < 3 else kaug)[0:64, S * head:S * (head + 1)]
                    eng = nc.scalar.copy if t < 3 else nc.vector.tensor_copy
                    eng(dst, qp[64 * sub:64 * sub + 64, :])

            # ---- v projection ----
            def v_proj():
                for sb in range(8):
                    vp = ps.tile([128, 384], F32, tag="big", name="vp")
                    if FP8_QKV:
                        for j in range(3):
                            nc.tensor.matmul(
                                vp[:],
                                _ap(xTp_sb, 2048 * j + 128 * sb,
                                    [[1024, 2], [1, 128]]),
                                _ap(wvp_sb, 768 * j, [[384, 2], [1, 384]]),
                                start=(j == 0), stop=(j == 2), perf_mode=DR)
                    else:
                        for ci in range(6):
                            nc.tensor.matmul(
                                vp[:],
                                xTp_sb[:, 1024 * ci + 128 * sb:
                                       1024 * ci + 128 * (sb + 1)],
                                wvp_sb[:, 384 * ci:384 * (ci + 1)],
                                start=(ci == 0), stop=(ci == 5))
                    dst = _ap(vps, 1536 * (sb // 2) + 768 * (sb % 2),
                              [[128, HPC], [1, 64]])
                    src = _ap(vp, 0, [[64, HPC], [1, 64]])
                    nc.vector.tensor_copy(dst, src)
                for j in range(4):
                    nc.gpsimd.memset(
                        _ap(vps, 1536 * j + 64, [[768, 2], [128, HPC]]), 1.0)
                    nc.gpsimd.memset(
                        _ap(vps, 1536 * j + 65, [[768, 2], [128, HPC], [1, 63]]),
                        0.0)

            # ---- direct band extraction for a head pair ----
            def band(p):
                for ax, tbl in ((0, rh2_sb), (1, rw2_sb)):
                    for qt in range(4):
                        bt = ps.tile([32, 512], F32, tag="band", name="bt",
                                     bufs=2)
                        for s8 in range(8):
                            s = 8 * qt + s8
                            lhsT = tbl[:, 31 - s:63 - s]
                            if ax == 0:
                                rhs = _app(qaug, 0, 64, 2048 * p + 32 * s,
                                           [[1024, 2], [1, 32]])
                            else:
                                rhs = _app(qaug, 0, 64, 2048 * p + s,
                                           [[1024, 2], [32, 32]])
                            nc.tensor.matmul(
                                _ap(bt, 32 * s8, [[256, 2], [1, 32]]),
                                lhsT, rhs, start=True, stop=True)
                        for hh in range(2):
                            i = 2 * p + hh
                            eng = nc.vector.tensor_copy
                            if ax == 0:
                                eng(qaug[64:96, S * i + 256 * qt:
                                         S * i + 256 * (qt + 1)],
                                    bt[:, 256 * hh:256 * (hh + 1)])
                            else:
                                dst = _app(qaug, 96, 32, S * i + 8 * qt,
                                           [[32, 32], [1, 8]])
                                src = _ap(bt, 256 * hh, [[1, 32], [32, 8]])
                                eng(dst, src)

            # ---- attention for one head ----
            def attn(i):
                av = ps.tile([128, S], F32, tag="av", name="av", bufs=1)
                for j in range(4):
                    e = small.tile([128, 2048], BF16, tag="et", name="et",
                                   bufs=3)
                    for kb2 in range(2):
                        kb = 2 * j + kb2
                        sc = ps.tile([128, S], F32, tag="big", name="sc")
                        for nh in range(2):
                            sl = 512 * nh
                            nc.tensor.matmul(
                                sc[:, sl:sl + 512],
                                kaug[:, S * i + 128 * kb:S * i + 128 * (kb + 1)],
                                qaug[:, S * i + sl:S * i + sl + 512],
                                start=True, stop=True)
                        nc.scalar.activation(
                            e[:, 1024 * kb2:1024 * (kb2 + 1)], sc[:],
                            mybir.ActivationFunctionType.Exp, scale=EXP_SCALE)
                        for nh in range(2):
                            sl = 512 * nh
                            nc.tensor.matmul(
                                av[:, sl:sl + 512],
                                _ap(vps, 1536 * j + 768 * kb2 + 128 * i,
                                    [[1, 128]]),
                                _ap(e, 1024 * kb2 + sl, [[1, 512]]),
                                start=(kb == 0), stop=(kb == 7))
                avs = small.tile([65, S], F32, tag="avs", name="avs", bufs=2)
                nc.vector.tensor_copy(avs[0:65, :], av[0:65, :])
                # single-lane reciprocal on (1, S) is ~9 cyc/elem; bounce the
                # row through an SBUF->SBUF DMA transpose to use 128 lanes
                rs_t = small.tile([128, 8], F32, tag="rs_t", name="rs_t",
                                  bufs=2)
                nc.sync.dma_start(rs_t[:], avs[64:65, :])
                rc_t = small.tile([128, 8], F32, tag="rc_t", name="rc_t",
                                  bufs=2)
                nc.vector.reciprocal(rc_t[:], rs_t[:])
                rec = small.tile([1, S], F32, tag="rec", name="rec", bufs=2)
                nc.sync.dma_start(rec[:], rc_t[:])
                rb = small.tile([64, S], F32, tag="rb", name="rb", bufs=2)
                nc.gpsimd.partition_broadcast(rb[:], rec[:])
                chunk, row = i // 2, (i % 2) * 64
                nc.vector.tensor_tensor(
                    ohT[row:row + 64, S * chunk:S * (chunk + 1)],
                    avs[0:64, :], rb[:], op=mybir.AluOpType.mult)

            # ---- schedule: stagger PE-only work between attention heads ----
            qk_oct(0); qk_oct(3)
            v_proj()
            band(0)
            qk_oct(1); qk_oct(4)
            attn(0)
            band(1)
            attn(1)
            qk_oct(2); qk_oct(5)
            attn(2)
            band(2)
            attn(3)
            attn(4)
            attn(5)

            # ---- output projection (bf16) ----
            for qb in range(8):
                pp = ps.tile([128, C], F32, tag="big", name="pp")
                for ci in range(3):
                    lhsT = ohT[:, S * ci + 128 * qb:S * ci + 128 * (qb + 1)]
                    nc.tensor.matmul(pp[:, 0:512], lhsT,
                                     wpr_sb[:, 768 * ci:768 * ci + 512],
                                     start=(ci == 0), stop=(ci == 2))
                    nc.tensor.matmul(pp[:, 512:768], lhsT,
                                     wpr_sb[:, 768 * ci + 512:768 * (ci + 1)],
                                     start=(ci == 0), stop=(ci == 2))
                pps = small.tile([128, C], F32, tag="pps", name="pps", bufs=2)
                (nc.scalar.copy if qb % 2 else nc.vector.tensor_copy)(
                    pps[:], pp[:])
                nc.sync.dma_start(out[128 * qb:128 * (qb + 1), :], pps[:])

    nc.compile()
    return nc


def shard_inputs(x, Wqkv, Wproj, rel_pos_h, rel_pos_w):
    """Build the 8 per-core input maps."""
    import ml_dtypes
    bf16 = ml_dtypes.bfloat16
    fp8 = ml_dtypes.float8_e4m3
    xdt = fp8 if FP8_QKV else bf16
    scale = HD ** (-0.5)
    x = np.asarray(x, dtype=np.float32)
    Wqkv = np.asarray(Wqkv, dtype=np.float32)
    Wproj = np.asarray(Wproj, dtype=np.float32)

    # flipped rel-pos tables, x64: rhTf[c, j] = 64 * rel_pos[62-j, c]
    rh2 = np.ascontiguousarray(
        (np.asarray(rel_pos_h, np.float32).T[:, ::-1] * 64.0)).astype(bf16)
    rw2 = np.ascontiguousarray(
        (np.asarray(rel_pos_w, np.float32).T[:, ::-1] * 64.0)).astype(bf16)

    # one-hot selector rows for kaug rows 64-127
    ohk = np.zeros((64, S), np.float32)
    kh = np.arange(S) // W
    kw = np.arange(S) % W
    ohk[kh, np.arange(S)] = 1.0
    ohk[32 + kw, np.arange(S)] = 1.0
    ohk = ohk.astype(bf16)

    def lay(a):
        # (768, M) -> SBUF image (128, 6M)
        M = a.shape[1]
        if FP8_QKV:
            # pair-interleaved (128, 3, 2, M) for DoubleRow
            r = a.reshape(3, 2, 128, M).transpose(2, 0, 1, 3)
        else:
            r = a.reshape(6, 128, M).transpose(1, 0, 2)
        return np.ascontiguousarray(r.reshape(128, 6 * M)).astype(xdt)

    in_maps = []
    for core in range(NCORES):
        b = core // 2
        h0 = (core % 2) * HPC
        xb = x[b].reshape(S, C)
        xT = np.ascontiguousarray(xb.T)
        wq = Wqkv[:, h0 * HD:(h0 + HPC) * HD] * 64.0
        wk = Wqkv[:, C + h0 * HD:C + (h0 + HPC) * HD] * (64.0 * scale)
        wqk = np.concatenate([wq, wk], axis=1)
        wv = Wqkv[:, 2 * C + h0 * HD:2 * C + (h0 + HPC) * HD] * 64.0
        wp = np.ascontiguousarray(
            Wproj[h0 * HD:(h0 + HPC) * HD, :] / 64.0).astype(bf16)
        in_maps.append({"xTp": lay(xT), "wqkp": lay(wqk), "wvp": lay(wv),
                        "wpr": wp, "rh2": rh2, "rw2": rw2, "ohk": ohk})
    return in_maps


_NC_CACHE = {}


def kernel(x, Wqkv, Wproj, bproj, rel_pos_h, rel_pos_w):
    if "nc" not in _NC_CACHE:
        _NC_CACHE["nc"] = build_program()
    nc = _NC_CACHE["nc"]
    in_maps = shard_inputs(x, Wqkv, Wproj, rel_pos_h, rel_pos_w)
    res = run_bass_kernel_spmd(nc, in_maps, list(range(NCORES)))
    bproj = np.asarray(bproj, dtype=np.float32)
    out = np.empty((B, H, W, C), dtype=np.float32)
    for b in range(B):
        acc = res.results[2 * b]["out"] + res.results[2 * b + 1]["out"] + bproj
        out[b] = acc.reshape(H, W, C)
    return out


# revision 12
# speedup vs baseline: 1.1515x; 1.1515x over previous
"""Trainium2 Bass kernel for ViTDet-style attention with decomposed
relative-position bias.

Problem shapes (hardcoded):
  x: (4, 32, 32, 768) f32, Wqkv: (768, 2304), Wproj: (768, 768),
  bproj: (768,), rel_pos_h/w: (63, 64).
  12 heads, head_dim 64, S = 32*32 = 1024.

Sharding: 48 (batch, head) pairs -> 6 heads per core, all of one batch per
core-pair. Each core computes its heads' attention and a partial output
projection (its heads' channel rows of Wproj); the host sums the two
partials per batch and adds bproj.

Device algorithm per core:
  - qkT = Wqk^T @ x^T and v = x @ Wv. With FP8_QKV these use fp8e4
    DoubleRow (two 128-row K tiles per pass, 2x); otherwise bf16.
    Weights are pre-scaled x64 (k additionally by 1/sqrt(hd)) so fp8 stays
    in normal range; the exp activation's scale=2^-12 folds it back out.
  - rel-pos bias band rows computed DIRECTLY per shift: for query row h the
    band tile rows are rhT_flip[:, 31-h:63-h]^T @ q64 — no 63-row table
    intermediate. Two heads per matmul (a matmul's PSUM extent must stay
    inside one 2KB bank), four 256-col quarters per axis.
  - scoresT (k x q) = kaug^T @ qaug in one K=128 bf16 matmul per tile:
    rows 0-63 k8/q64, 64-95 one-hot(kh)/bandH, 96-127 one-hot(kw)/bandW.
  - e = exp(2^-12 * scores) on ScalarE; av accumulates over k blocks with
    v blocks padded to 128 stationary cols (64 v + ones + 63 zeros); row 64
    of av is the softmax denominator via the ones column.
  - normalize: av->SBUF f32 copy (frees PSUM fast), DVE
    reciprocal_approx_fast on the denominator row (plain reciprocal is ~9
    cyc/elem and single-lane here), gpsimd partition-broadcast, DVE mult.
  - partial = out_headsT^T @ (Wproj/64) in bf16, PSUM->SBUF->DRAM.
"""

import numpy as np

import concourse.bass as bass
import concourse.bacc as bacc
import concourse.mybir as mybir
import concourse.tile as tile
from concourse.bass_utils import run_bass_kernel_spmd

F32 = mybir.dt.float32
BF16 = mybir.dt.bfloat16
FP8 = mybir.dt.float8e4

NH = 12          # total heads
C = 768
HD = 64
H = W = 32
S = H * W        # 1024
B = 4
NCORES = 8
HPC = NH * B // NCORES   # heads per core = 6
EXP_SCALE = float(2.0 ** -12)

FP8_QKV = False  # fp8 DoubleRow for the qk/v projections (adds ~1.7e-2 err)


def _ap(t, off, dims):
    return bass.AP(t.tensor, t[:].offset + off, [t[:].ap[0]] + dims)


def _app(t, p0, psz, off, dims):
    # AP with partition offset/size override
    base = t[p0:p0 + psz, :]
    return bass.AP(t.tensor, base.offset + off, [base.ap[0]] + dims)


def build_program():
    nc = bacc.Bacc("TRN2", target_bir_lowering=False, debug=False)

    XDT = FP8 if FP8_QKV else BF16
    xTp = nc.declare_dram_parameter("xTp", [128, 6144], XDT, isOutput=False)
    wqkp = nc.declare_dram_parameter("wqkp", [128, 4608], XDT, isOutput=False)
    wvp = nc.declare_dram_parameter("wvp", [128, 2304], XDT, isOutput=False)
    wpr = nc.declare_dram_parameter("wpr", [384, 768], BF16, isOutput=False)
    rh2 = nc.declare_dram_parameter("rh2", [64, 63], BF16, isOutput=False)
    rw2 = nc.declare_dram_parameter("rw2", [64, 63], BF16, isOutput=False)
    ohk = nc.declare_dram_parameter("ohk", [64, 1024], BF16, isOutput=False)
    out = nc.declare_dram_parameter("out", [S, C], F32, isOutput=True)

    with tile.TileContext(nc) as tc:
        with (
            tc.tile_pool(name="persist", bufs=1) as persist,
            tc.tile_pool(name="ps", bufs=2, space="PSUM") as ps,
            tc.tile_pool(name="small", bufs=2) as small,
        ):
            # ---- persistent SBUF loads: split per chunk and interleaved so
            # the first qk matmul (needs only xT/wqk chunk 0) starts ~1us in
            xTp_sb = persist.tile([128, 6144], XDT, tag="xTp", name="xTp_sb")
            wqkp_sb = persist.tile([128, 4608], XDT, tag="wqkp", name="wqkp_sb")
            nch = 3 if FP8_QKV else 6
            xw, ww = 6144 // nch, 4608 // nch
            for ci in range(nch):
                nc.sync.dma_start(xTp_sb[:, xw * ci:xw * (ci + 1)],
                                  xTp[:, xw * ci:xw * (ci + 1)])
                nc.sync.dma_start(wqkp_sb[:, ww * ci:ww * (ci + 1)],
                                  wqkp[:, ww * ci:ww * (ci + 1)])
            rh2_sb = persist.tile([64, 63], BF16, tag="rh2", name="rh2_sb")
            nc.sync.dma_start(rh2_sb[:], rh2[:, :])
            rw2_sb = persist.tile([64, 63], BF16, tag="rw2", name="rw2_sb")
            nc.sync.dma_start(rw2_sb[:], rw2[:, :])

            # qaug/kaug: per head i at cols [1024i, 1024(i+1)):
            #   rows 0-63 q64 / k8, 64-95 bandH / onehot(kh), 96-127 bandW /
            #   onehot(kw)
            qaug = persist.tile([128, HPC * S], BF16, tag="qaug", name="qaug")
            kaug = persist.tile([128, HPC * S], BF16, tag="kaug", name="kaug")
            nc.sync.dma_start(kaug[64:128, 0:S], ohk[:, :])
            for i in range(1, HPC):
                nc.sync.dma_start(kaug[64:128, S * i:S * (i + 1)],
                                  kaug[64:128, 0:S])

            wvp_sb = persist.tile([128, 2304], XDT, tag="wvp", name="wvp_sb")
            vw = 2304 // nch
            for ci in range(nch):
                nc.sync.dma_start(wvp_sb[:, vw * ci:vw * (ci + 1)],
                                  wvp[:, vw * ci:vw * (ci + 1)])
            wpr_sb = persist.tile([128, 2304], BF16, tag="wpr", name="wpr_sb")
            nc.sync.dma_start(
                wpr_sb[:],
                bass.AP(wpr, 0, [[768, 128], [128 * 768, 3], [1, 768]]))

            # v in bf16 pair layout: vps[p, j, i, h, d] with j = k-block pair,
            # i = which block of the pair, h = head, d = 128 cols
            # (64 v + ones + 63 zeros)
            vps = persist.tile([128, 4 * 2 * HPC * 128], BF16, tag="vps",
                               name="vps")
            ohT = persist.tile([128, 3 * S], BF16, tag="ohT", name="ohT")

            DR = mybir.MatmulPerfMode.DoubleRow

            # ---- qk projection ----
            def qk_oct(t):
                # octile t: rows 128t..128t+128 of qk output; t<3 -> q64,
                # t>=3 -> k8; heads (2(t%3), 2(t%3)+1)
                qp = ps.tile([128, S], F32, tag="big", name="qp")
                if FP8_QKV:
                    for j in range(3):
                        for nh in range(2):
                            sl = 512 * nh
                            nc.tensor.matmul(
                                qp[:, sl:sl + 512],
                                _ap(wqkp_sb, 1536 * j + 128 * t,
                                    [[768, 2], [1, 128]]),
                                _ap(xTp_sb, 2048 * j + sl, [[1024, 2], [1, 512]]),
                                start=(j == 0), stop=(j == 2), perf_mode=DR)
                else:
                    for ci in range(6):
                        for nh in range(2):
                            sl = 512 * nh
                            nc.tensor.matmul(
                                qp[:, sl:sl + 512],
                                wqkp_sb[:, 768 * ci + 128 * t:
                                        768 * ci + 128 * (t + 1)],
                                xTp_sb[:, 1024 * ci + sl:1024 * ci + sl + 512],
                                start=(ci == 0), stop=(ci == 5))
                for sub in range(2):
                    head = (t % 3) * 2 + sub
                    dst = (qaug if t < 3 else kaug)[0:64, S * head:S * (head + 1)]
                    eng = nc.scalar.copy if t < 3 else nc.vector.tensor_copy
                    eng(dst, qp[64 * sub:64 * sub + 64, :])

            # ---- v projection ----
            def v_proj():
                for sb in range(8):
                    vp = ps.tile([128, 384], F32, tag="big", name="vp")
                    if FP8_QKV:
                        for j in range(3):
                            nc.tensor.matmul(
                                vp[:],
                                _ap(xTp_sb, 2048 * j + 128 * sb,
                                    [[1024, 2], [1, 128]]),
                                _ap(wvp_sb, 768 * j, [[384, 2], [1, 384]]),
                                start=(j == 0), stop=(j == 2), perf_mode=DR)
                    else:
                        for ci in range(6):
                            nc.tensor.matmul(
                                vp[:],
                                xTp_sb[:, 1024 * ci + 128 * sb:
                                       1024 * ci + 128 * (sb + 1)],
                                wvp_sb[:, 384 * ci:384 * (ci + 1)],
                                start=(ci == 0), stop=(ci == 5))
                    dst = _ap(vps, 1536 * (sb // 2) + 768 * (sb % 2),
                              [[128, HPC], [1, 64]])
                    src = _ap(vp, 0, [[64, HPC], [1, 64]])
                    nc.vector.tensor_copy(dst, src)
                for j in range(4):
                    nc.gpsimd.memset(
                        _ap(vps, 1536 * j + 64, [[768, 2], [128, HPC]]), 1.0)
                    nc.gpsimd.memset(
                        _ap(vps, 1536 * j + 65, [[768, 2], [128, HPC], [1, 63]]),
                        0.0)

            # ---- direct band extraction for a head pair ----
            def band(p):
                for ax, tbl in ((0, rh2_sb), (1, rw2_sb)):
                    for qt in range(4):
                        bt = ps.tile([32, 512], F32, tag="band", name="bt",
                                     bufs=2)
                        for s8 in range(8):
                            s = 8 * qt + s8
                            lhsT = tbl[:, 31 - s:63 - s]
                            if ax == 0:
                                rhs = _app(qaug, 0, 64, 2048 * p + 32 * s,
                                           [[1024, 2], [1, 32]])
                            else:
                                rhs = _app(qaug, 0, 64, 2048 * p + s,
                                           [[1024, 2], [32, 32]])
                            nc.tensor.matmul(
                                _ap(bt, 32 * s8, [[256, 2], [1, 32]]),
                                lhsT, rhs, start=True, stop=True)
                        for hh in range(2):
                            i = 2 * p + hh
                            eng = nc.vector.tensor_copy
                            if ax == 0:
                                eng(qaug[64:96, S * i + 256 * qt:
                                         S * i + 256 * (qt + 1)],
                                    bt[:, 256 * hh:256 * (hh + 1)])
                            else:
                                dst = _app(qaug, 96, 32, S * i + 8 * qt,
                                           [[32, 32], [1, 8]])
                                src = _ap(bt, 256 * hh, [[1, 32], [32, 8]])
                                eng(dst, src)

            # ---- attention for one head ----
            def attn(i):
                av = ps.tile([128, S], F32, tag="av", name="av", bufs=1)
                for j in range(4):
                    e = small.tile([128, 2048], BF16, tag="et", name="et",
                                   bufs=3)
                    for kb2 in range(2):
                        kb = 2 * j + kb2
                        sc = ps.tile([128, S], F32, tag="big", name="sc")
                        for nh in range(2):
                            sl = 512 * nh
                            nc.tensor.matmul(
                                sc[:, sl:sl + 512],
                                kaug[:, S * i + 128 * kb:S * i + 128 * (kb + 1)],
                                qaug[:, S * i + sl:S * i + sl + 512],
                                start=True, stop=True)
                        nc.scalar.activation(
                            e[:, 1024 * kb2:1024 * (kb2 + 1)], sc[:],
                            mybir.ActivationFunctionType.Exp, scale=EXP_SCALE)
                        for nh in range(2):
                            sl = 512 * nh
                            nc.tensor.matmul(
                                av[:, sl:sl + 512],
                                _ap(vps, 1536 * j + 768 * kb2 + 128 * i,
                                    [[1, 128]]),
                                _ap(e, 1024 * kb2 + sl, [[1, 512]]),
                                start=(kb == 0), stop=(kb == 7))
                avs = small.tile([65, S], F32, tag="avs", name="avs", bufs=2)
                nc.vector.tensor_copy(avs[0:65, :], av[0:65, :])
                # single-lane reciprocal on (1, S) is ~9 cyc/elem; bounce the
                # row through an SBUF->SBUF DMA transpose to use 128 lanes
                rs_t = small.tile([128, 8], F32, tag="rs_t", name="rs_t",
                                  bufs=2)
                nc.sync.dma_start(rs_t[:], avs[64:65, :])
                rc_t = small.tile([128, 8], F32, tag="rc_t", name="rc_t",
                                  bufs=2)
                nc.vector.reciprocal(rc_t[:], rs_t[:])
                rec = small.tile([1, S], F32, tag="rec", name="rec", bufs=2)
                nc.sync.dma_start(rec[:], rc_t[:])
                rb = small.tile([64, S], F32, tag="rb", name="rb", bufs=2)
                nc.gpsimd.partition_broadcast(rb[:], rec[:])
                chunk, row = i // 2, (i % 2) * 64
                nc.vector.tensor_tensor(
                    ohT[row:row + 64, S * chunk:S * (chunk + 1)],
                    avs[0:64, :], rb[:], op=mybir.AluOpType.mult)

            # ---- schedule: stagger PE-only work between attention heads ----
            qk_oct(0); qk_oct(3)
            v_proj()
            band(0)
            qk_oct(1); qk_oct(4)
            attn(0)
            band(1)
            attn(1)
            qk_oct(2); qk_oct(5)
            attn(2)
            band(2)
            attn(3)
            attn(4)
            attn(5)

            # ---- output projection (bf16) ----
            for qb in range(8):
                pp = ps.tile([128, C], F32, tag="big", name="pp")
                for ci in range(3):
                    lhsT = ohT[:, S * ci + 128 * qb:S * ci + 128 * (qb + 1)]
                    nc.tensor.matmul(pp[:, 0:512], lhsT,
                                     wpr_sb[:, 768 * ci:768 * ci + 512],
                                     start=(ci == 0), stop=(ci == 2))
                    nc.tensor.matmul(pp[:, 512:768], lhsT,
                                     wpr_sb[:, 768 * ci + 512:768 * (ci + 1)],
                                     start=(ci == 0), stop=(ci == 2))
                pps = small.tile([128, C], F32, tag="pps", name="pps", bufs=2)
                (nc.scalar.copy if qb % 2 else nc.vector.tensor_copy)(
                    pps[:], pp[:])
                nc.sync.dma_start(out[128 * qb:128 * (qb + 1), :], pps[:])

    nc.compile()
    return nc


def shard_inputs(x, Wqkv, Wproj, rel_pos_h, rel_pos_w):
    """Build the 8 per-core input maps."""
    import ml_dtypes
    bf16 = ml_dtypes.bfloat16
    fp8 = ml_dtypes.float8_e4m3
    xdt = fp8 if FP8_QKV else bf16
    scale = HD ** (-0.5)
    x = np.asarray(x, dtype=np.float32)
    Wqkv = np.asarray(Wqkv, dtype=np.float32)
    Wproj = np.asarray(Wproj, dtype=np.float32)

    # flipped rel-pos tables, x64: rhTf[c, j] = 64 * rel_pos[62-j, c]
    rh2 = np.ascontiguousarray(
        (np.asarray(rel_pos_h, np.float32).T[:, ::-1] * 64.0)).astype(bf16)
    rw2 = np.ascontiguousarray(
        (np.asarray(rel_pos_w, np.float32).T[:, ::-1] * 64.0)).astype(bf16)

    # one-hot selector rows for kaug rows 64-127
    ohk = np.zeros((64, S), np.float32)
    kh = np.arange(S) // W
    kw = np.arange(S) % W
    ohk[kh, np.arange(S)] = 1.0
    ohk[32 + kw, np.arange(S)] = 1.0
    ohk = ohk.astype(bf16)

    def lay(a):
        # (768, M) -> SBUF image (128, 6M)
        M = a.shape[1]
        if FP8_QKV:
            # pair-interleaved (128, 3, 2, M) for DoubleRow
            r = a.reshape(3, 2, 128, M).transpose(2, 0, 1, 3)
        else:
            r = a.reshape(6, 128, M).transpose(1, 0, 2)
        return np.ascontiguousarray(r.reshape(128, 6 * M)).astype(xdt)

    in_maps = []
    for core in range(NCORES):
        b = core // 2
        h0 = (core % 2) * HPC
        xb = x[b].reshape(S, C)
        xT = np.ascontiguousarray(xb.T)
        wq = Wqkv[:, h0 * HD:(h0 + HPC) * HD] * 64.0
        wk = Wqkv[:, C + h0 * HD:C + (h0 + HPC) * HD] * (64.0 * scale)
        wqk = np.concatenate([wq, wk], axis=1)
        wv = Wqkv[:, 2 * C + h0 * HD:2 * C + (h0 + HPC) * HD] * 64.0
        wp = np.ascontiguousarray(
            Wproj[h0 * HD:(h0 + HPC) * HD, :] / 64.0).astype(bf16)
        in_maps.append({"xTp": lay(xT), "wqkp": lay(wqk), "wvp": lay(wv),
                        "wpr": wp, "rh2": rh2, "rw2": rw2, "ohk": ohk})
    return in_maps


_NC_CACHE = {}


def kernel(x, Wqkv, Wproj, bproj, rel_pos_h, rel_pos_w):
    if "nc" not in _NC_CACHE:
        _NC_CACHE["nc"] = build_program()
    nc = _NC_CACHE["nc"]
    in_maps = shard_inputs(x, Wqkv, Wproj, rel_pos_h, rel_pos_w)
    res = run_bass_kernel_spmd(nc, in_maps, list(range(NCORES)))
    bproj = np.asarray(bproj, dtype=np.float32)
    out = np.empty((B, H, W, C), dtype=np.float32)
    for b in range(B):
        acc = res.results[2 * b]["out"] + res.results[2 * b + 1]["out"] + bproj
        out[b] = acc.reshape(H, W, C)
    return out


# revision 19
# speedup vs baseline: 1.1783x; 1.0233x over previous
"""Trainium2 Bass kernel for ViTDet-style attention with decomposed
relative-position bias.

Problem shapes (hardcoded):
  x: (4, 32, 32, 768) f32, Wqkv: (768, 2304), Wproj: (768, 768),
  bproj: (768,), rel_pos_h/w: (63, 64).
  12 heads, head_dim 64, S = 32*32 = 1024.

Sharding: 48 (batch, head) pairs -> 6 heads per core, all of one batch per
core-pair. Each core computes its heads' attention and a partial output
projection (its heads' channel rows of Wproj); the host sums the two
partials per batch and adds bproj.

Device algorithm per core:
  - qkT = Wqk^T @ x^T and v = x @ Wv. With FP8_QKV these use fp8e4
    DoubleRow (two 128-row K tiles per pass, 2x); otherwise bf16.
    Weights are pre-scaled x64 (k additionally by 1/sqrt(hd)) so fp8 stays
    in normal range; the exp activation's scale=2^-12 folds it back out.
  - rel-pos bias band rows computed DIRECTLY per shift: for query row h the
    band tile rows are rhT_flip[:, 31-h:63-h]^T @ q64 — no 63-row table
    intermediate. Two heads per matmul (a matmul's PSUM extent must stay
    inside one 2KB bank), four 256-col quarters per axis.
  - scoresT (k x q) = kaug^T @ qaug in one K=128 bf16 matmul per tile:
    rows 0-63 k8/q64, 64-95 one-hot(kh)/bandH, 96-127 one-hot(kw)/bandW.
  - e = exp(2^-12 * scores) on ScalarE; av accumulates over k blocks with
    v blocks padded to 128 stationary cols (64 v + ones + 63 zeros); row 64
    of av is the softmax denominator via the ones column.
  - normalize: av->SBUF f32 copy (frees PSUM fast), DVE
    reciprocal_approx_fast on the denominator row (plain reciprocal is ~9
    cyc/elem and single-lane here), gpsimd partition-broadcast, DVE mult.
  - partial = out_headsT^T @ (Wproj/64) in bf16, PSUM->SBUF->DRAM.
"""

import numpy as np

import concourse.bass as bass
import concourse.bacc as bacc
import concourse.mybir as mybir
import concourse.tile as tile
from concourse.bass_utils import run_bass_kernel_spmd

F32 = mybir.dt.float32
BF16 = mybir.dt.bfloat16
FP8 = mybir.dt.float8e4

NH = 12          # total heads
C = 768
HD = 64
H = W = 32
S = H * W        # 1024
B = 4
NCORES = 8
HPC = NH * B // NCORES   # heads per core = 6
EXP_SCALE = float(2.0 ** -12)

FP8_QKV = False  # fp8 DoubleRow for the qk/v projections (adds ~1.7e-2 err)


def _ap(t, off, dims):
    return bass.AP(t.tensor, t[:].offset + off, [t[:].ap[0]] + dims)


def _app(t, p0, psz, off, dims):
    # AP with partition offset/size override
    base = t[p0:p0 + psz, :]
    return bass.AP(t.tensor, base.offset + off, [base.ap[0]] + dims)


def build_program():
    nc = bacc.Bacc("TRN2", target_bir_lowering=False, debug=False)

    XDT = FP8 if FP8_QKV else BF16
    xTp = nc.declare_dram_parameter("xTp", [128, 6144], XDT, isOutput=False)
    wqkp = nc.declare_dram_parameter("wqkp", [128, 4608], XDT, isOutput=False)
    wvp = nc.declare_dram_parameter("wvp", [128, 2304], XDT, isOutput=False)
    wpr = nc.declare_dram_parameter("wpr", [384, 768], BF16, isOutput=False)
    rh2 = nc.declare_dram_parameter("rh2", [64, 63], BF16, isOutput=False)
    rw2 = nc.declare_dram_parameter("rw2", [64, 63], BF16, isOutput=False)
    ohk = nc.declare_dram_parameter("ohk", [64, 1024], BF16, isOutput=False)
    out = nc.declare_dram_parameter("out", [S, C], BF16, isOutput=True)

    with tile.TileContext(nc) as tc:
        with (
            tc.tile_pool(name="persist", bufs=1) as persist,
            tc.tile_pool(name="ps", bufs=2, space="PSUM") as ps,
            tc.tile_pool(name="small", bufs=2) as small,
        ):
            # ---- persistent SBUF loads: split per chunk and interleaved so
            # the first qk matmul (needs only xT/wqk chunk 0) starts ~1us in
            xTp_sb = persist.tile([128, 6144], XDT, tag="xTp", name="xTp_sb")
            wqkp_sb = persist.tile([128, 4608], XDT, tag="wqkp", name="wqkp_sb")
            nch = 3 if FP8_QKV else 6
            xw, ww = 6144 // nch, 4608 // nch
            for ci in range(nch):
                nc.sync.dma_start(xTp_sb[:, xw * ci:xw * (ci + 1)],
                                  xTp[:, xw * ci:xw * (ci + 1)])
                nc.sync.dma_start(wqkp_sb[:, ww * ci:ww * (ci + 1)],
                                  wqkp[:, ww * ci:ww * (ci + 1)])
            rh2_sb = persist.tile([64, 63], BF16, tag="rh2", name="rh2_sb")
            nc.sync.dma_start(rh2_sb[:], rh2[:, :])
            rw2_sb = persist.tile([64, 63], BF16, tag="rw2", name="rw2_sb")
            nc.sync.dma_start(rw2_sb[:], rw2[:, :])

            # qaug/kaug: per head i at cols [1024i, 1024(i+1)):
            #   rows 0-63 q64 / k8, 64-95 bandH / onehot(kh), 96-127 bandW /
            #   onehot(kw)
            qaug = persist.tile([128, HPC * S], BF16, tag="qaug", name="qaug")
            kaug = persist.tile([128, HPC * S], BF16, tag="kaug", name="kaug")
            nc.sync.dma_start(kaug[64:128, 0:S], ohk[:, :])
            for i in range(1, HPC):
                nc.sync.dma_start(kaug[64:128, S * i:S * (i + 1)],
                                  kaug[64:128, 0:S])

            wvp_sb = persist.tile([128, 2304], XDT, tag="wvp", name="wvp_sb")
            vw = 2304 // nch
            for ci in range(nch):
                nc.sync.dma_start(wvp_sb[:, vw * ci:vw * (ci + 1)],
                                  wvp[:, vw * ci:vw * (ci + 1)])
            wpr_sb = persist.tile([128, 2304], BF16, tag="wpr", name="wpr_sb")
            nc.sync.dma_start(
                wpr_sb[:],
                bass.AP(wpr, 0, [[768, 128], [128 * 768, 3], [1, 768]]))

            # v in bf16 pair layout: vps[p, j, i, h, d] with j = k-block pair,
            # i = which block of the pair, h = head, d = 128 cols
            # (64 v + ones + 63 zeros)
            vps = persist.tile([128, 4 * 2 * HPC * 128], BF16, tag="vps",
                               name="vps")
            ohT = persist.tile([128, 3 * S], BF16, tag="ohT", name="ohT")

            DR = mybir.MatmulPerfMode.DoubleRow

            # ---- qk projection ----
            def qk_oct(t):
                # octile t: rows 128t..128t+128 of qk output; t<3 -> q64,
                # t>=3 -> k8; heads (2(t%3), 2(t%3)+1)
                qp = ps.tile([128, S], F32, tag="big", name="qp")
                if FP8_QKV:
                    for j in range(3):
                        for nh in range(2):
                            sl = 512 * nh
                            nc.tensor.matmul(
                                qp[:, sl:sl + 512],
                                _ap(wqkp_sb, 1536 * j + 128 * t,
                                    [[768, 2], [1, 128]]),
                                _ap(xTp_sb, 2048 * j + sl, [[1024, 2], [1, 512]]),
                                start=(j == 0), stop=(j == 2), perf_mode=DR)
                else:
                    for ci in range(6):
                        for nh in range(2):
                            sl = 512 * nh
                            nc.tensor.matmul(
                                qp[:, sl:sl + 512],
                                wqkp_sb[:, 768 * ci + 128 * t:
                                        768 * ci + 128 * (t + 1)],
                                xTp_sb[:, 1024 * ci + sl:1024 * ci + sl + 512],
                                start=(ci == 0), stop=(ci == 5))
                for sub in range(2):
                    head = (t % 3) * 2 + sub
                    dst = (qaug if t < 3 else kaug)[0:64, S * head:S * (head + 1)]
                    eng = nc.scalar.copy if t < 3 else nc.vector.tensor_copy
                    eng(dst, qp[64 * sub:64 * sub + 64, :])

            # ---- v projection ----
            def v_proj():
                for sb in range(8):
                    vp = ps.tile([128, 384], F32, tag="big", name="vp")
                    if FP8_QKV:
                        for j in range(3):
                            nc.tensor.matmul(
                                vp[:],
                                _ap(xTp_sb, 2048 * j + 128 * sb,
                                    [[1024, 2], [1, 128]]),
                                _ap(wvp_sb, 768 * j, [[384, 2], [1, 384]]),
                                start=(j == 0), stop=(j == 2), perf_mode=DR)
                    else:
                        for ci in range(6):
                            nc.tensor.matmul(
                                vp[:],
                                xTp_sb[:, 1024 * ci + 128 * sb:
                                       1024 * ci + 128 * (sb + 1)],
                                wvp_sb[:, 384 * ci:384 * (ci + 1)],
                                start=(ci == 0), stop=(ci == 5))
                    dst = _ap(vps, 1536 * (sb // 2) + 768 * (sb % 2),
                              [[128, HPC], [1, 64]])
                    src = _ap(vp, 0, [[64, HPC], [1, 64]])
                    nc.vector.tensor_copy(dst, src)
                for j in range(4):
                    nc.gpsimd.memset(
                        _ap(vps, 1536 * j + 64, [[768, 2], [128, HPC]]), 1.0)
                    nc.gpsimd.memset(
                        _ap(vps, 1536 * j + 65, [[768, 2], [128, HPC], [1, 63]]),
                        0.0)

            # ---- direct band extraction for a head pair ----
            def band(p):
                for ax, tbl in ((0, rh2_sb), (1, rw2_sb)):
                    for qt in range(4):
                        bt = ps.tile([32, 512], F32, tag="band", name="bt",
                                     bufs=2)
                        for s8 in range(8):
                            s = 8 * qt + s8
                            lhsT = tbl[:, 31 - s:63 - s]
                            if ax == 0:
                                rhs = _app(qaug, 0, 64, 2048 * p + 32 * s,
                                           [[1024, 2], [1, 32]])
                            else:
                                rhs = _app(qaug, 0, 64, 2048 * p + s,
                                           [[1024, 2], [32, 32]])
                            nc.tensor.matmul(
                                _ap(bt, 32 * s8, [[256, 2], [1, 32]]),
                                lhsT, rhs, start=True, stop=True)
                        for hh in range(2):
                            i = 2 * p + hh
                            eng = nc.vector.tensor_copy
                            if ax == 0:
                                eng(qaug[64:96, S * i + 256 * qt:
                                         S * i + 256 * (qt + 1)],
                                    bt[:, 256 * hh:256 * (hh + 1)])
                            else:
                                dst = _app(qaug, 96, 32, S * i + 8 * qt,
                                           [[32, 32], [1, 8]])
                                src = _ap(bt, 256 * hh, [[1, 32], [32, 8]])
                                eng(dst, src)

            # ---- attention for one head ----
            def attn(i):
                av = ps.tile([128, S], F32, tag="av", name="av", bufs=1)
                for j in range(4):
                    e = small.tile([128, 2048], BF16, tag="et", name="et",
                                   bufs=3)
                    for kb2 in range(2):
                        kb = 2 * j + kb2
                        sc = ps.tile([128, S], F32, tag="big", name="sc")
                        for nh in range(2):
                            sl = 512 * nh
                            nc.tensor.matmul(
                                sc[:, sl:sl + 512],
                                kaug[:, S * i + 128 * kb:S * i + 128 * (kb + 1)],
                                qaug[:, S * i + sl:S * i + sl + 512],
                                start=True, stop=True)
                        nc.scalar.activation(
                            e[:, 1024 * kb2:1024 * (kb2 + 1)], sc[:],
                            mybir.ActivationFunctionType.Exp, scale=EXP_SCALE)
                        for nh in range(2):
                            sl = 512 * nh
                            nc.tensor.matmul(
                                av[:, sl:sl + 512],
                                _ap(vps, 1536 * j + 768 * kb2 + 128 * i,
                                    [[1, 128]]),
                                _ap(e, 1024 * kb2 + sl, [[1, 512]]),
                                start=(kb == 0), stop=(kb == 7))
                avs = small.tile([65, S], F32, tag="avs", name="avs", bufs=2)
                nc.vector.tensor_copy(avs[0:65, :], av[0:65, :])
                # single-lane reciprocal on (1, S) is ~9 cyc/elem; bounce the
                # row through an SBUF->SBUF DMA transpose to use 128 lanes
                rs_t = small.tile([128, 8], F32, tag="rs_t", name="rs_t",
                                  bufs=2)
                nc.sync.dma_start(rs_t[:], avs[64:65, :])
                rc_t = small.tile([128, 8], F32, tag="rc_t", name="rc_t",
                                  bufs=2)
                nc.vector.reciprocal(rc_t[:], rs_t[:])
                rec = small.tile([1, S], F32, tag="rec", name="rec", bufs=2)
                nc.sync.dma_start(rec[:], rc_t[:])
                rb = small.tile([64, S], F32, tag="rb", name="rb", bufs=2)
                nc.gpsimd.partition_broadcast(rb[:], rec[:])
                chunk, row = i // 2, (i % 2) * 64
                nc.vector.tensor_tensor(
                    ohT[row:row + 64, S * chunk:S * (chunk + 1)],
                    avs[0:64, :], rb[:], op=mybir.AluOpType.mult)

            # ---- schedule: stagger PE-only work between attention heads ----
            qk_oct(0); qk_oct(3)
            v_proj()
            band(0)
            qk_oct(1); qk_oct(4)
            attn(0)
            band(1)
            attn(1)
            qk_oct(2); qk_oct(5)
            attn(2)
            band(2)
            attn(3)
            attn(4)
            attn(5)

            # ---- output projection (bf16) ----
            for qb in range(8):
                pp = ps.tile([128, C], F32, tag="big", name="pp")
                for ci in range(3):
                    lhsT = ohT[:, S * ci + 128 * qb:S * ci + 128 * (qb + 1)]
                    nc.tensor.matmul(pp[:, 0:512], lhsT,
                                     wpr_sb[:, 768 * ci:768 * ci + 512],
                                     start=(ci == 0), stop=(ci == 2))
                    nc.tensor.matmul(pp[:, 512:768], lhsT,
                                     wpr_sb[:, 768 * ci + 512:768 * (ci + 1)],
                                     start=(ci == 0), stop=(ci == 2))
                pps = small.tile([128, C], BF16, tag="pps", name="pps", bufs=2)
                (nc.scalar.copy if qb % 2 else nc.vector.tensor_copy)(
                    pps[:], pp[:])
                nc.sync.dma_start(out[128 * qb:128 * (qb + 1), :], pps[:])

    nc.compile()
    return nc


def shard_inputs(x, Wqkv, Wproj, rel_pos_h, rel_pos_w):
    """Build the 8 per-core input maps."""
    import ml_dtypes
    bf16 = ml_dtypes.bfloat16
    fp8 = ml_dtypes.float8_e4m3
    xdt = fp8 if FP8_QKV else bf16
    scale = HD ** (-0.5)
    x = np.asarray(x, dtype=np.float32)
    Wqkv = np.asarray(Wqkv, dtype=np.float32)
    Wproj = np.asarray(Wproj, dtype=np.float32)

    # flipped rel-pos tables, x64: rhTf[c, j] = 64 * rel_pos[62-j, c]
    rh2 = np.ascontiguousarray(
        (np.asarray(rel_pos_h, np.float32).T[:, ::-1] * 64.0)).astype(bf16)
    rw2 = np.ascontiguousarray(
        (np.asarray(rel_pos_w, np.float32).T[:, ::-1] * 64.0)).astype(bf16)

    # one-hot selector rows for kaug rows 64-127
    ohk = np.zeros((64, S), np.float32)
    kh = np.arange(S) // W
    kw = np.arange(S) % W
    ohk[kh, np.arange(S)] = 1.0
    ohk[32 + kw, np.arange(S)] = 1.0
    ohk = ohk.astype(bf16)

    def lay(a):
        # (768, M) -> SBUF image (128, 6M)
        M = a.shape[1]
        if FP8_QKV:
            # pair-interleaved (128, 3, 2, M) for DoubleRow
            r = a.reshape(3, 2, 128, M).transpose(2, 0, 1, 3)
        else:
            r = a.reshape(6, 128, M).transpose(1, 0, 2)
        return np.ascontiguousarray(r.reshape(128, 6 * M)).astype(xdt)

    in_maps = []
    for core in range(NCORES):
        b = core // 2
        h0 = (core % 2) * HPC
        xb = x[b].reshape(S, C)
        xT = np.ascontiguousarray(xb.T)
        wq = Wqkv[:, h0 * HD:(h0 + HPC) * HD] * 64.0
        wk = Wqkv[:, C + h0 * HD:C + (h0 + HPC) * HD] * (64.0 * scale)
        wqk = np.concatenate([wq, wk], axis=1)
        wv = Wqkv[:, 2 * C + h0 * HD:2 * C + (h0 + HPC) * HD] * 64.0
        wp = np.ascontiguousarray(
            Wproj[h0 * HD:(h0 + HPC) * HD, :] / 64.0).astype(bf16)
        in_maps.append({"xTp": lay(xT), "wqkp": lay(wqk), "wvp": lay(wv),
                        "wpr": wp, "rh2": rh2, "rw2": rw2, "ohk": ohk})
    return in_maps


_NC_CACHE = {}


def kernel(x, Wqkv, Wproj, bproj, rel_pos_h, rel_pos_w):
    if "nc" not in _NC_CACHE:
        _NC_CACHE["nc"] = build_program()
    nc = _NC_CACHE["nc"]
    in_maps = shard_inputs(x, Wqkv, Wproj, rel_pos_h, rel_pos_w)
    res = run_bass_kernel_spmd(nc, in_maps, list(range(NCORES)))
    bproj = np.asarray(bproj, dtype=np.float32)
    out = np.empty((B, H, W, C), dtype=np.float32)
    for b in range(B):
        acc = (np.asarray(res.results[2 * b]["out"], np.float32)
               + np.asarray(res.results[2 * b + 1]["out"], np.float32) + bproj)
        out[b] = acc.reshape(H, W, C)
    return out
